# revision 16
# baseline (speedup 1.0000x reference)
"""DirectAU loss kernel for Trainium2, SPMD over 8 NeuronCores.

Math (see reference):
  user_e = user_table[user_id]; pos_e = item_table[pos_id]   (B=8192, D=64)
  align  = mean_i ||un_i - pn_i||^2 = 2 - (2/B) sum_i <un_i, pn_i>
  unif(x)= log( (sum_{i<j} exp(-4 + 4 <xn_i, xn_j>)) / npairs )
  out    = align + 0.5*(unif(user_e) + unif(pos_e))

Strategy:
  - Tables replicated to all cores; batch sharded by chunks of 1024 with a
    per-core *rotated* id layout so the SPMD program is identical and all
    per-core variation lives in the int32 index data.
  - Each core computes its diagonal Gram block at weight 1/2 (folded into the
    exp bias: exp(4s-4+ln(1/2))), full blocks at chunk distance 1..3, and two
    512x512 quadrants of the distance-4 block (halves swapped on cores 4..7 in
    host index prep) => every unordered pair covered exactly once.
  - Per core: one indirect-DMA gather per table (5120 rows), normalize rows
    (DVE square/reduce + Newton rsqrt, no ACT table switches), PE-transpose to
    [64, 5120], then 72 matmuls (K=64) -> PSUM -> ACT exp in place with
    accum_out row-sums into an accumulator tile. Host sums the 8x[128,64]
    partials and applies the closed-form log/align finalization (pure unshard).
"""

import math
import os

import numpy as np

import concourse.bacc as bacc
import concourse.bass as bass
import concourse.mybir as mybir
import concourse.tile as tile
from concourse import bass_utils
from concourse.masks import make_identity

B = 8192
DIM = 64
NROWS = 100000
NCORES = 8
CHUNK = 1024  # batch rows per core
NSLOT = 5120  # column slots per core: 4 full chunks + dist-4 chunk
NBAND = NSLOT // 128  # 40 bands of 128 rows
OWN_BANDS = CHUNK // 128  # 8
LN_HALF = math.log(0.5)
F32 = mybir.dt.float32
I32 = mybir.dt.int32

# accumulator column map
N_CHUNKS_PER_RT = 3  # D, O1, O2
U_COL0 = 0  # 24 cols
P_COL0 = 24  # 24 cols
ALIGN_COL = 48
ACC_W = 64


def _emit_rsqrt(nc, pool, x_ap, out_ap, n, tag):
    """out = 1/sqrt(x) on the vector engine (bit-hack seed + 3 Newton steps)."""
    MAGIC = 0x5F3759DF
    op = mybir.AluOpType
    ti = pool.tile([128, n], I32, tag=f"{tag}_ti")
    nc.vector.tensor_scalar(
        out=ti[:], in0=x_ap.bitcast(I32), scalar1=1, scalar2=None,
        op0=op.logical_shift_right,
    )
    yi = pool.tile([128, n], I32, tag=f"{tag}_yi")
    # MAGIC - t == (t ^ -1) + (MAGIC + 1); split: ISA can't mix bitwise+arith
    nc.vector.tensor_scalar(
        out=yi[:], in0=ti[:], scalar1=-1, scalar2=None, op0=op.bitwise_xor
    )
    nc.vector.tensor_scalar(
        out=yi[:], in0=yi[:], scalar1=MAGIC + 1, scalar2=None, op0=op.add
    )
    xh = pool.tile([128, n], F32, tag=f"{tag}_xh")
    nc.vector.tensor_scalar(
        out=xh[:], in0=x_ap, scalar1=-0.5, scalar2=None, op0=op.mult
    )
    cur = yi[:].bitcast(F32)
    for it in range(3):
        t2 = pool.tile([128, n], F32, tag=f"{tag}_t2")
        nc.vector.tensor_mul(out=t2[:], in0=cur, in1=cur)
        nc.vector.tensor_mul(out=t2[:], in0=t2[:], in1=xh[:])
        nc.vector.tensor_scalar(
            out=t2[:], in0=t2[:], scalar1=1.5, scalar2=None, op0=op.add
        )
        if it == 2:
            dst_ap = out_ap
        else:
            yt = pool.tile([128, n], F32, tag=f"{tag}_y{it}", name=f"{tag}_y{it}")
            dst_ap = yt[:]
        nc.vector.tensor_mul(out=dst_ap, in0=cur, in1=t2[:])
        cur = dst_ap
    return cur


def _body(tc, ut, it_, uidx, pidx, acc):
    nc = tc.nc
    op = mybir.AluOpType
    with (
        tc.tile_pool(name="persist", bufs=1) as P,
        tc.tile_pool(name="work", bufs=2) as W,
        tc.tile_pool(name="ps", bufs=2, space="PSUM") as PS,
    ):
        ident = P.tile([128, 128], F32, tag="ident")
        make_identity(nc, ident[:])

        idx_sb = P.tile([128, 2 * NBAND], I32, tag="idx")
        nc.sync.dma_start(out=idx_sb[:, 0:NBAND], in_=uidx)
        nc.sync.dma_start(out=idx_sb[:, NBAND : 2 * NBAND], in_=pidx)

        accw = P.tile([128, ACC_W], F32, tag="accw")
        nc.gpsimd.memset(accw[:], 0.0)

        bias_o = P.tile([128, 1], F32, tag="bias_o")
        nc.gpsimd.memset(bias_o[:], -4.0)
        bias_d = P.tile([128, 1], F32, tag="bias_d")
        nc.gpsimd.memset(bias_d[:], -4.0 + LN_HALF)

        gath = [
            P.tile([128, NSLOT * DIM // 128], F32, tag=f"gath{t}", name=f"gath{t}")
            for t in (0, 1)
        ]
        # gath[t] layout: [128, NBAND, DIM]; slot s=(c*128+p) -> gath[p, c, :]
        xnT = [
            P.tile([64, NSLOT], F32, tag=f"xnT{t}", name=f"xnT{t}") for t in (0, 1)
        ]
        # norm^2 layout: cols [0:8]=u-own, [8:16]=p-own, [16:48]=u-rest, [48:80]=p-rest
        nsq = P.tile([128, 2 * NBAND], F32, tag="nsq")
        rinv = P.tile([128, 2 * NBAND], F32, tag="rinv")

        tables = [ut, it_]
        # gather one band (128 rows) per instruction; own bands first so the
        # diagonal blocks can start while the rest streams in.
        band_order = [(t, c) for c in range(OWN_BANDS) for t in (0, 1)] + [
            (t, c) for c in range(OWN_BANDS, NBAND) for t in (0, 1)
        ]
        for t, c in band_order:
            nc.gpsimd.indirect_dma_start(
                out=gath[t][:, c * DIM : (c + 1) * DIM],
                out_offset=None,
                in_=tables[t],
                in_offset=bass.IndirectOffsetOnAxis(
                    ap=idx_sb[:, t * NBAND + c : t * NBAND + c + 1], axis=0
                ),
            )

        stop_after = os.environ.get("K_STOP_AFTER", "")
        if stop_after == "gather":
            nc.vector.tensor_copy(out=accw[:], in_=gath[0][:, 0:ACC_W])
            nc.sync.dma_start(out=acc, in_=accw[:])
            return

        # normalization: norms^2 per (slot) via square+reduce, grouped so the
        # own-chunk group finishes early.
        def nsq_col(t, c):
            return (8 * t + c) if c < OWN_BANDS else (16 + 32 * t + (c - OWN_BANDS))

        groups = [
            (0, 0, OWN_BANDS), (1, 0, OWN_BANDS),
            (0, OWN_BANDS, NBAND), (1, OWN_BANDS, NBAND),
        ]
        for t, c0, c1 in groups:
            nb = c1 - c0
            sq = W.tile([128, nb * DIM], F32, tag="sq")
            g3 = gath[t][:].rearrange("p (c d) -> p c d", d=DIM)[:, c0:c1, :]
            nc.vector.tensor_tensor(out=sq[:], in0=g3, in1=g3, op=op.mult)
            nc.vector.tensor_reduce(
                out=nsq[:, nsq_col(t, c0) : nsq_col(t, c0) + nb],
                in_=sq[:].rearrange("p (c d) -> p c d", d=DIM),
                axis=mybir.AxisListType.X,
                op=op.add,
            )
        # rsqrt in two chains: own cols [0:16] first, rest [16:80]
        _emit_rsqrt(nc, W, nsq[:, 0:16], rinv[:, 0:16], 16, "nwa")
        _emit_rsqrt(nc, W, nsq[:, 16:80], rinv[:, 16:80], 64, "nwb")

        # scale rows in place: gath[t][:, c, :] *= rinv[:, nsq_col(t, c)]
        for t, c0, c1 in groups:
            nb = c1 - c0
            g3 = gath[t][:].rearrange("p (c d) -> p c d", d=DIM)[:, c0:c1, :]
            r3 = (
                rinv[:, nsq_col(t, c0) : nsq_col(t, c0) + nb]
                .rearrange("p (c o) -> p c o", o=1)
                .to_broadcast([128, nb, DIM])
            )
            nc.vector.tensor_tensor(out=g3, in0=g3, in1=r3, op=op.mult)

        if stop_after == "normalize":
            nc.vector.tensor_copy(out=accw[:], in_=gath[0][:, 0:ACC_W])
            nc.sync.dma_start(out=acc, in_=accw[:])
            return

        # align: sum_i <un_i, pn_i> over own chunk (bands 0..7)
        # (tensor_tensor_reduce crashes HW through this path; use mul+reduce)
        al_sc = W.tile([128, OWN_BANDS * DIM], F32, tag="alsc")
        nc.vector.tensor_mul(
            out=al_sc[:],
            in0=gath[0][:, 0 : OWN_BANDS * DIM],
            in1=gath[1][:, 0 : OWN_BANDS * DIM],
        )
        nc.vector.tensor_reduce(
            out=accw[:, ALIGN_COL : ALIGN_COL + 1],
            in_=al_sc[:],
            axis=mybir.AxisListType.X,
            op=op.add,
        )

        # transpose to xnT[t] = [64, NSLOT], 4 bands per PSUM trip
        for t in (0, 1):
            for g in range(NBAND // 4):
                pt = PS.tile([128, 2048], F32, tag="ps")
                for k in range(4):
                    c = g * 4 + k
                    nc.tensor.transpose(
                        out=pt[0:64, k * 128 : (k + 1) * 128],
                        in_=gath[t][:, c * DIM : (c + 1) * DIM],
                        identity=ident[:],
                    )
                nc.vector.tensor_copy(
                    out=xnT[t][:, g * 512 : (g + 1) * 512], in_=pt[0:64, 0:512]
                )

        if stop_after == "transpose":
            nc.vector.tensor_copy(out=accw[0:64, :], in_=xnT[0][:, 0:ACC_W])
            nc.sync.dma_start(out=acc, in_=accw[:])
            return

        # Gram blocks: per (table, row-tile): 9 col-tiles of 512
        #   D  = {0,1}   bias -4+ln(1/2)   (diagonal chunk, weight 1/2)
        #   O1 = {2,3,4,5} bias -4
        #   O2 = {6,7,8} bias -4; col-tile 8 is the dist-4 quadrant
        for t in (0, 1):
            for rt in range(8):
                lhs = xnT[t][:, rt * 128 : (rt + 1) * 128]
                chunks = [
                    ([0, 1], bias_d),
                    ([2, 3, 4, 5], bias_o),
                    ([6, 7, 8], bias_o),
                ]
                for ci, (tiles_, bias_t) in enumerate(chunks):
                    pt = PS.tile([128, 2048], F32, tag="ps")
                    w = len(tiles_) * 512
                    for k, j in enumerate(tiles_):
                        cs = j * 512 if j < 8 else (4096 if rt < 4 else 4608)
                        nc.tensor.matmul(
                            out=pt[:, k * 512 : (k + 1) * 512],
                            lhsT=lhs,
                            rhs=xnT[t][:, cs : cs + 512],
                            start=True,
                            stop=True,
                        )
                    col = (U_COL0 if t == 0 else P_COL0) + rt * N_CHUNKS_PER_RT + ci
                    nc.scalar.activation(
                        out=pt[:, 0:w],
                        in_=pt[:, 0:w],
                        func=mybir.ActivationFunctionType.Exp,
                        bias=bias_t[:],
                        scale=4.0,
                        accum_out=accw[:, col : col + 1],
                    )

        nc.sync.dma_start(out=acc, in_=accw[:])


def _build():
    nc = bacc.Bacc(
        "TRN2",
        target_bir_lowering=False,
        debug=False,
        enable_asserts=False,
        num_devices=NCORES,
    )
    ut = nc.dram_tensor("ut", [NROWS, DIM], F32, kind="ExternalInput").ap()
    it_ = nc.dram_tensor("it", [NROWS, DIM], F32, kind="ExternalInput").ap()
    uidx = nc.dram_tensor("uidx", [128, NBAND], I32, kind="ExternalInput").ap()
    pidx = nc.dram_tensor("pidx", [128, NBAND], I32, kind="ExternalInput").ap()
    acc = nc.dram_tensor("acc", [128, ACC_W], F32, kind="ExternalOutput").ap()
    with tile.TileContext(nc) as tc:
        _body(tc, ut, it_, uidx, pidx, acc)
    nc.compile()
    return nc


_PROG = None


def _get_prog():
    global _PROG
    if _PROG is None:
        _PROG = _build()
    return _PROG


def _core_idx(ids, m):
    """Rotated per-core slot ids -> [128, NBAND] int32 band-major index tile."""
    ch = ids.reshape(NCORES, CHUNK)
    segs = [ch[(m + d) % NCORES] for d in range(4)]
    c4 = ch[(m + 4) % NCORES]
    d4 = c4 if m < 4 else np.concatenate([c4[512:], c4[:512]])
    slots = np.concatenate(segs + [d4]).astype(np.int32)
    assert slots.shape == (NSLOT,)
    return np.ascontiguousarray(slots.reshape(NBAND, 128).T)


def _make_in_maps(user_id, pos_id, user_table, item_table):
    ut = np.ascontiguousarray(np.asarray(user_table, dtype=np.float32))
    it_ = np.ascontiguousarray(np.asarray(item_table, dtype=np.float32))
    uid = np.asarray(user_id).astype(np.int32)
    pid = np.asarray(pos_id).astype(np.int32)
    return [
        {
            "ut": ut,
            "it": it_,
            "uidx": _core_idx(uid, m),
            "pidx": _core_idx(pid, m),
        }
        for m in range(NCORES)
    ]


def _finalize(accs):
    """accs: list of [128, ACC_W] per core -> scalar loss."""
    a = np.stack([np.asarray(x, dtype=np.float64) for x in accs])  # [8,128,64]
    s_u = a[:, :, U_COL0 : U_COL0 + 24].sum()
    s_p = a[:, :, P_COL0 : P_COL0 + 24].sum()
    s_al = a[:, :, ALIGN_COL].sum()
    npairs = B * (B - 1) // 2
    pair_u = s_u - B / 2.0
    pair_p = s_p - B / 2.0
    unif = 0.5 * (np.log(pair_u / npairs) + np.log(pair_p / npairs))
    align = 2.0 - (2.0 / B) * s_al
    return np.asarray(align + unif, dtype=np.float32)


def _run(in_maps, trace=False, **kw):
    nc = _get_prog()
    return bass_utils.run_bass_kernel_spmd(
        nc, in_maps, core_ids=list(range(NCORES)), trace=trace, **kw
    )


def kernel(user_id, pos_id, neg_id=None, user_table=None, item_table=None):
    in_maps = _make_in_maps(user_id, pos_id, user_table, item_table)
    res = _run(in_maps, trace=False)
    return _finalize([res.results[m]["acc"] for m in range(NCORES)])


def _install_profile_hook():
    """The image's antenv lacks axon_hooks; shim it so trace=True can reach
    the NTFF profiler in libaxon_pjrt.so (same mechanism trn_boot uses)."""
    import sys
    import types

    if "antenv.axon_hooks" in sys.modules:
        return
    import antenv
    from trn_agent_boot.trn_boot import _ntff_profile_via_ctypes

    mod = types.ModuleType("antenv.axon_hooks")
    holder = [None]
    mod.set_axon_ntff_profile_hook = lambda h: holder.__setitem__(0, h)
    mod.get_axon_ntff_profile_hook = lambda: holder[0]
    sys.modules["antenv.axon_hooks"] = mod
    antenv.axon_hooks = mod
    mod.set_axon_ntff_profile_hook(
        _ntff_profile_via_ctypes("/opt/axon/libaxon_pjrt.so")
    )
    # no bucket filesystem in this container
    bass_utils.upload_artifacts = lambda tmpdir: ""


def run_profiled(user_id, pos_id, neg_id=None, user_table=None, item_table=None, **kw):
    _install_profile_hook()
    in_maps = _make_in_maps(user_id, pos_id, user_table, item_table)
    res = _run(in_maps, trace=True, **kw)
    out = _finalize([res.results[m]["acc"] for m in range(NCORES)])
    return out, res


# revision 17
# speedup vs baseline: 1.8624x; 1.8624x over previous
"""DirectAU loss kernel for Trainium2, SPMD over 8 NeuronCores.

Math (see reference):
  user_e = user_table[user_id]; pos_e = item_table[pos_id]   (B=8192, D=64)
  align  = mean_i ||un_i - pn_i||^2 = 2 - (2/B) sum_i <un_i, pn_i>
  unif(x)= log( (sum_{i<j} exp(-4 + 4 <xn_i, xn_j>)) / npairs )
  out    = align + 0.5*(unif(user_e) + unif(pos_e))

Strategy (v2):
  - Batch sharded by chunks of 1024; tables replicated. Each core indirect-DMA
    gathers only its OWN chunk's rows (16 x 128-row gathers), normalizes them
    (DVE square/reduce + Newton rsqrt -> no ACT table switches), transposes via
    PE to xnT_own [64,1024] in bf16, then AllGathers the normalized transposed
    blocks of both tables (2MB bf16 across 8 cores).
  - Each core's Gram columns are the 4.5 chunks it owns under a balanced
    triangular schedule: diag chunk at weight 1/2 (folded into the exp bias:
    exp(4s-4+ln .5)), chunks at distance 1..3 in full, and two 512x512
    quadrants of the distance-4 chunk (halves swapped on cores 4..7). The
    rotated column layout is re-gathered from the AG buffer with 10 indirect
    DMAs of whole [64,1024]/[64,512] blocks whose indices are per-core DATA,
    so the SPMD program is identical on all cores.
  - Diag blocks read only local data, so their matmul+exp overlap the
    collective. 72 bf16 matmuls (K=64) per table -> PSUM -> ACT exp in place
    with accum_out row-sums into an accumulator tile. Host sums the 8x[128,64]
    partials and applies the closed-form log/align finalization (pure unshard).
"""

import math
import os

import numpy as np

import concourse.bacc as bacc
import concourse.bass as bass
import concourse.mybir as mybir
import concourse.tile as tile
from concourse import bass_utils
from concourse.masks import make_identity

B = 8192
DIM = 64
NROWS = 100000
NCORES = 8
CHUNK = 1024  # batch rows per core
OWN_BANDS = CHUNK // 128  # 8
NREST = 4096  # re-gathered column slots: d1,d2,d3 full + two d4 halves
LN_HALF = math.log(0.5)
F32 = mybir.dt.float32
BF16 = mybir.dt.bfloat16
I32 = mybir.dt.int32

# accumulator column map
N_CHUNKS_PER_RT = 3  # D, O1, O2
U_COL0 = 0  # 24 cols
P_COL0 = 24  # 24 cols
ALIGN_COL = 48
ACC_W = 64


def _emit_rsqrt(nc, pool, x_ap, out_ap, n, tag):
    """out = 1/sqrt(x) on the vector engine (bit-hack seed + 3 Newton steps)."""
    MAGIC = 0x5F3759DF
    op = mybir.AluOpType
    ti = pool.tile([128, n], I32, tag=f"{tag}_ti")
    nc.vector.tensor_scalar(
        out=ti[:], in0=x_ap.bitcast(I32), scalar1=1, scalar2=None,
        op0=op.logical_shift_right,
    )
    yi = pool.tile([128, n], I32, tag=f"{tag}_yi")
    # MAGIC - t == (t ^ -1) + (MAGIC + 1); split: ISA can't mix bitwise+arith
    nc.vector.tensor_scalar(
        out=yi[:], in0=ti[:], scalar1=-1, scalar2=None, op0=op.bitwise_xor
    )
    nc.vector.tensor_scalar(
        out=yi[:], in0=yi[:], scalar1=MAGIC + 1, scalar2=None, op0=op.add
    )
    xh = pool.tile([128, n], F32, tag=f"{tag}_xh")
    nc.vector.tensor_scalar(
        out=xh[:], in0=x_ap, scalar1=-0.5, scalar2=None, op0=op.mult
    )
    cur = yi[:].bitcast(F32)
    for it in range(3):
        t2 = pool.tile([128, n], F32, tag=f"{tag}_t2")
        nc.vector.tensor_mul(out=t2[:], in0=cur, in1=cur)
        nc.vector.tensor_mul(out=t2[:], in0=t2[:], in1=xh[:])
        nc.vector.tensor_scalar(
            out=t2[:], in0=t2[:], scalar1=1.5, scalar2=None, op0=op.add
        )
        if it == 2:
            dst_ap = out_ap
        else:
            yt = pool.tile([128, n], F32, tag=f"{tag}_y{it}", name=f"{tag}_y{it}")
            dst_ap = yt[:]
        nc.vector.tensor_mul(out=dst_ap, in0=cur, in1=t2[:])
        cur = dst_ap
    return cur


def _body(tc, ut, it_, uidx, pidx, ridx, acc):
    nc = tc.nc
    op = mybir.AluOpType
    with (
        tc.tile_pool(name="persist", bufs=1) as P,
        tc.tile_pool(name="work", bufs=2) as W,
        tc.tile_pool(name="ps", bufs=2, space="PSUM") as PS,
        tc.tile_pool(name="dram", bufs=1, space="DRAM") as DP,
    ):
        ident = P.tile([128, 128], F32, tag="ident")
        make_identity(nc, ident[:])

        idx_sb = P.tile([128, 2 * OWN_BANDS], I32, tag="idx")
        nc.sync.dma_start(out=idx_sb[:, 0:OWN_BANDS], in_=uidx)
        nc.sync.dma_start(out=idx_sb[:, OWN_BANDS : 2 * OWN_BANDS], in_=pidx)
        ridx_sb = P.tile([64, 10], I32, tag="ridx")
        nc.sync.dma_start(out=ridx_sb[:], in_=ridx)

        accw = P.tile([128, ACC_W], F32, tag="accw")
        nc.gpsimd.memset(accw[:], 0.0)

        bias_o = P.tile([128, 1], F32, tag="bias_o")
        nc.gpsimd.memset(bias_o[:], -4.0)
        bias_d = P.tile([128, 1], F32, tag="bias_d")
        nc.gpsimd.memset(bias_d[:], -4.0 + LN_HALF)

        # own-chunk gathered rows, [128, band, DIM] band-major slots
        gath = [
            P.tile([128, OWN_BANDS * DIM], F32, tag=f"gath{t}", name=f"gath{t}")
            for t in (0, 1)
        ]
        xnT = [
            P.tile([64, CHUNK], BF16, tag=f"xnT{t}", name=f"xnT{t}") for t in (0, 1)
        ]
        rhs = [
            P.tile([64, NREST], BF16, tag=f"rhs{t}", name=f"rhs{t}") for t in (0, 1)
        ]
        nsq = P.tile([128, 2 * OWN_BANDS], F32, tag="nsq")
        rinv = P.tile([128, 2 * OWN_BANDS], F32, tag="rinv")

        tables = [ut, it_]
        for c in range(OWN_BANDS):
            for t in (0, 1):
                nc.gpsimd.indirect_dma_start(
                    out=gath[t][:, c * DIM : (c + 1) * DIM],
                    out_offset=None,
                    in_=tables[t],
                    in_offset=bass.IndirectOffsetOnAxis(
                        ap=idx_sb[:, t * OWN_BANDS + c : t * OWN_BANDS + c + 1],
                        axis=0,
                    ),
                )

        # normalization
        for t in (0, 1):
            sq = W.tile([128, OWN_BANDS * DIM], F32, tag="sq")
            g3 = gath[t][:].rearrange("p (c d) -> p c d", d=DIM)
            nc.vector.tensor_tensor(out=sq[:], in0=g3, in1=g3, op=op.mult)
            nc.vector.tensor_reduce(
                out=nsq[:, t * OWN_BANDS : (t + 1) * OWN_BANDS],
                in_=sq[:].rearrange("p (c d) -> p c d", d=DIM),
                axis=mybir.AxisListType.X,
                op=op.add,
            )
        _emit_rsqrt(nc, W, nsq[:], rinv[:], 2 * OWN_BANDS, "nw")
        for t in (0, 1):
            g3 = gath[t][:].rearrange("p (c d) -> p c d", d=DIM)
            r3 = (
                rinv[:, t * OWN_BANDS : (t + 1) * OWN_BANDS]
                .rearrange("p (c o) -> p c o", o=1)
                .to_broadcast([128, OWN_BANDS, DIM])
            )
            nc.vector.tensor_tensor(out=g3, in0=g3, in1=r3, op=op.mult)

        # align: sum_i <un_i, pn_i> over own chunk
        al_sc = W.tile([128, OWN_BANDS * DIM], F32, tag="alsc")
        nc.vector.tensor_mul(out=al_sc[:], in0=gath[0][:], in1=gath[1][:])
        nc.vector.tensor_reduce(
            out=accw[:, ALIGN_COL : ALIGN_COL + 1],
            in_=al_sc[:],
            axis=mybir.AxisListType.X,
            op=op.add,
        )

        # transpose own rows -> xnT (bf16), 4 bands per PSUM trip
        for t in (0, 1):
            for g in range(OWN_BANDS // 4):
                pt = PS.tile([128, 2048], F32, tag="ps")
                for k in range(4):
                    c = g * 4 + k
                    nc.tensor.transpose(
                        out=pt[0:64, k * 128 : (k + 1) * 128],
                        in_=gath[t][:, c * DIM : (c + 1) * DIM],
                        identity=ident[:],
                    )
                nc.vector.tensor_copy(
                    out=xnT[t][:, g * 512 : (g + 1) * 512], in_=pt[0:64, 0:512]
                )

        # AllGather both tables' normalized transposed blocks (bf16)
        ag_src = DP.tile([2, 64, CHUNK], BF16, tag="ag_src")
        ag_dst = DP.tile([NCORES, 2, 64, CHUNK], BF16, tag="ag_dst",
                         addr_space="Shared")
        for t in (0, 1):
            nc.sync.dma_start(out=ag_src[t, :, :], in_=xnT[t][:])
        nc.gpsimd.collective_compute(
            "AllGather",
            mybir.AluOpType.bypass,
            ins=[ag_src[:]],
            outs=[ag_dst[:]],
            replica_groups=[list(range(NCORES))],
        )

        # Gram blocks. col-tile j of row-tile rt:
        #   j in {0,1}: diag chunk (local xnT), bias -4+ln(1/2)
        #   j in 2..7:  rhs[:, (j-2)*512 ...]           (d1..d3)
        #   j == 8:     quadrant: rhs[:, 3072:3584] if rt<4 else [3584:4096]
        def rhs_ap(t, rt, j):
            if j < 2:
                return xnT[t][:, j * 512 : (j + 1) * 512]
            if j < 8:
                return rhs[t][:, (j - 2) * 512 : (j - 1) * 512]
            cs = 3072 if rt < 4 else 3584
            return rhs[t][:, cs : cs + 512]

        def emit_chunk(t, rt, ci, tiles_, bias_t):
            lhs = xnT[t][:, rt * 128 : (rt + 1) * 128]
            pt = PS.tile([128, 2048], F32, tag="ps", name=f"mm{t}_{rt}_{ci}")
            w = len(tiles_) * 512
            for k, j in enumerate(tiles_):
                nc.tensor.matmul(
                    out=pt[:, k * 512 : (k + 1) * 512],
                    lhsT=lhs,
                    rhs=rhs_ap(t, rt, j),
                    start=True,
                    stop=True,
                )
            col = (U_COL0 if t == 0 else P_COL0) + rt * N_CHUNKS_PER_RT + ci
            nc.scalar.activation(
                out=pt[:, 0:w],
                in_=pt[:, 0:w],
                func=mybir.ActivationFunctionType.Exp,
                bias=bias_t[:],
                scale=4.0,
                accum_out=accw[:, col : col + 1],
            )

        # diag chunks first: local-only, overlap the collective
        for t in (0, 1):
            for rt in range(8):
                emit_chunk(t, rt, 0, [0, 1], bias_d)

        # re-gather rotated columns from the AG buffer (indices are data)
        ag_rows = ag_dst[:].rearrange("r t d n -> (r t d) n")  # [1024, 1024]
        ag_half = ag_dst[:].rearrange("r t d (h n) -> (r t d h) n", h=2)  # [2048, 512]
        for t in (0, 1):
            for k in range(3):  # d1, d2, d3
                nc.gpsimd.indirect_dma_start(
                    out=rhs[t][:, k * CHUNK : (k + 1) * CHUNK],
                    out_offset=None,
                    in_=ag_rows,
                    in_offset=bass.IndirectOffsetOnAxis(
                        ap=ridx_sb[:, t * 5 + k : t * 5 + k + 1], axis=0
                    ),
                )
            for h in range(2):  # d4 quadrant halves (swap encoded in host data)
                nc.gpsimd.indirect_dma_start(
                    out=rhs[t][:, 3072 + h * 512 : 3072 + (h + 1) * 512],
                    out_offset=None,
                    in_=ag_half,
                    in_offset=bass.IndirectOffsetOnAxis(
                        ap=ridx_sb[:, t * 5 + 3 + h : t * 5 + 4 + h], axis=0
                    ),
                )

        # off-diagonal chunks
        for t in (0, 1):
            for rt in range(8):
                emit_chunk(t, rt, 1, [2, 3, 4, 5], bias_o)
                emit_chunk(t, rt, 2, [6, 7, 8], bias_o)

        nc.sync.dma_start(out=acc, in_=accw[:])


def _build():
    nc = bacc.Bacc(
        "TRN2",
        target_bir_lowering=False,
        debug=False,
        enable_asserts=False,
        num_devices=NCORES,
    )
    ut = nc.dram_tensor("ut", [NROWS, DIM], F32, kind="ExternalInput").ap()
    it_ = nc.dram_tensor("it", [NROWS, DIM], F32, kind="ExternalInput").ap()
    uidx = nc.dram_tensor("uidx", [128, OWN_BANDS], I32, kind="ExternalInput").ap()
    pidx = nc.dram_tensor("pidx", [128, OWN_BANDS], I32, kind="ExternalInput").ap()
    ridx = nc.dram_tensor("ridx", [64, 10], I32, kind="ExternalInput").ap()
    acc = nc.dram_tensor("acc", [128, ACC_W], F32, kind="ExternalOutput").ap()
    with tile.TileContext(nc) as tc:
        _body(tc, ut, it_, uidx, pidx, ridx, acc)
    nc.compile()
    return nc


_PROG = None


def _get_prog():
    global _PROG
    if _PROG is None:
        _PROG = _build()
    return _PROG


def _own_idx(ids, m):
    """Own-chunk ids -> [128, OWN_BANDS] band-major index tile."""
    ch = ids.reshape(NCORES, CHUNK)[m].astype(np.int32)
    return np.ascontiguousarray(ch.reshape(OWN_BANDS, 128).T)


def _ridx(m):
    """Re-gather row indices into the AG buffer, [64, 10] int32.

    Column t*5+k: k in 0..2 -> full chunk at distance k+1 (rows of the
    [8*2*64, 1024] view); k in 3..4 -> d4 quadrant halves (rows of the
    [8*2*64*2, 512] view), halves swapped for cores m >= 4.
    """
    d = np.arange(64)
    cols = []
    for t in (0, 1):
        for k in range(3):
            r = (m + 1 + k) % NCORES
            cols.append((r * 2 + t) * 64 + d)
        r4 = (m + 4) % NCORES
        for hi in range(2):
            h = hi if m < 4 else 1 - hi
            cols.append(((r4 * 2 + t) * 64 + d) * 2 + h)
    return np.ascontiguousarray(np.stack(cols, axis=1).astype(np.int32))


def _make_in_maps(user_id, pos_id, user_table, item_table):
    ut = np.ascontiguousarray(np.asarray(user_table, dtype=np.float32))
    it_ = np.ascontiguousarray(np.asarray(item_table, dtype=np.float32))
    uid = np.asarray(user_id).astype(np.int32)
    pid = np.asarray(pos_id).astype(np.int32)
    return [
        {
            "ut": ut,
            "it": it_,
            "uidx": _own_idx(uid, m),
            "pidx": _own_idx(pid, m),
            "ridx": _ridx(m),
        }
        for m in range(NCORES)
    ]


def _finalize(accs):
    """accs: list of [128, ACC_W] per core -> scalar loss."""
    a = np.stack([np.asarray(x, dtype=np.float64) for x in accs])  # [8,128,64]
    s_u = a[:, :, U_COL0 : U_COL0 + 24].sum()
    s_p = a[:, :, P_COL0 : P_COL0 + 24].sum()
    s_al = a[:, :, ALIGN_COL].sum()
    npairs = B * (B - 1) // 2
    pair_u = s_u - B / 2.0
    pair_p = s_p - B / 2.0
    unif = 0.5 * (np.log(pair_u / npairs) + np.log(pair_p / npairs))
    align = 2.0 - (2.0 / B) * s_al
    return np.asarray(align + unif, dtype=np.float32)


def _run(in_maps, trace=False, **kw):
    nc = _get_prog()
    return bass_utils.run_bass_kernel_spmd(
        nc, in_maps, core_ids=list(range(NCORES)), trace=trace, **kw
    )


def kernel(user_id, pos_id, neg_id=None, user_table=None, item_table=None):
    in_maps = _make_in_maps(user_id, pos_id, user_table, item_table)
    res = _run(in_maps, trace=False)
    return _finalize([res.results[m]["acc"] for m in range(NCORES)])


def _install_profile_hook():
    """The image's antenv lacks axon_hooks; shim it so trace=True can reach
    the NTFF profiler in libaxon_pjrt.so (same mechanism trn_boot uses)."""
    import sys
    import types

    if "antenv.axon_hooks" in sys.modules:
        return
    import antenv
    from trn_agent_boot.trn_boot import _ntff_profile_via_ctypes

    mod = types.ModuleType("antenv.axon_hooks")
    holder = [None]
    mod.set_axon_ntff_profile_hook = lambda h: holder.__setitem__(0, h)
    mod.get_axon_ntff_profile_hook = lambda: holder[0]
    sys.modules["antenv.axon_hooks"] = mod
    antenv.axon_hooks = mod
    mod.set_axon_ntff_profile_hook(
        _ntff_profile_via_ctypes("/opt/axon/libaxon_pjrt.so")
    )
    # no bucket filesystem in this container
    bass_utils.upload_artifacts = lambda tmpdir: ""


def run_profiled(user_id, pos_id, neg_id=None, user_table=None, item_table=None, **kw):
    _install_profile_hook()
    in_maps = _make_in_maps(user_id, pos_id, user_table, item_table)
    res = _run(in_maps, trace=True, **kw)
    out = _finalize([res.results[m]["acc"] for m in range(NCORES)])
    return out, res
